# revision 37
# baseline (speedup 1.0000x reference)
"""
Single-head causal attention on 8 Trainium2 NeuronCores.

Problem: embeddings [8, 2048, 1024] fp32, Wq/Wk/Wv [1024, 128] fp32.
    q,k,v = x @ W{q,k,v};  wei = softmax(mask(q k^T * C^-0.5));  out = wei @ v

Sharding: pure data-parallel - one batch element per core, no collectives.

Host-side prep per core (numpy, layout/precision only - all FLOPs stay on
device): cast x and W to fp16 and pack [w | x^T] so that each SBUF
partition's entire input is CONTIGUOUS in DRAM, chunk-major:
    packed[p] = [ wv | wq | wk (1024 each) | x_ch0 | x_ch1 | x_ch2 | x_ch3 ]
This turns every input DMA into 128 descriptors of 2-8 KB (vs 1 KB rows
for a naive x^T load), lifting per-queue DMA throughput from the
~63 GB/s descriptor-rate limit to ~200+ GB/s, so the whole input lands
in ~14 us instead of drip-feeding over 50 us.  The two HWDGE queues are
loaded in consumption order (wq + x chunk 0 first, then wv, wk, and the
remaining x chunks split across both queues).

Per-core device kernel (matmul operands fp16, fp32 PSUM accumulation):
  - 16 PE warmup matmuls on junk SBUF (gated only on one memset) while
    the input DMAs land: the PE pstate ramp + HAM full-clock grant need
    ~4.5 us of CONTINUOUS execution and a gap resets the grant timer,
    so the warmup bridges seamlessly into the first projection
  - per q-chunk ch (natural order 0..3):
      Q^T = Wq^T x^T on PE (N=512 cols, accumulated over C in PSUM),
      then the OFF-DIAGONAL S^T tiles (they need only prior chunks' K/V)
      interleaved around the V^T / K^T projections - this spreads the
      ACT exp load into the projections' shadow; without it the whole
      chunk's exp burst lands after the last projection and ACT becomes
      the bottleneck of the kernel's final stretch
      v natural [T,H] from V^T via 4 PE transposes (128x128 fp16)
      then the 4 diagonal tiles (valid q-range only, N = 512-128*d)
      per 128-key tile j:
        S^T_j = K_j^T.T @ Q^T_chunk      (PE -> PSUM fp32)
        P^T_j = exp(S^T_j / 32)          (ACT, PSUM->SBUF fp16; no
                                          max-sub: |S/32| <~ 2.5, safe)
        causal triangle on the diagonal block (gpsimd affine_select)
        A_chunk += P^T_j                 (DVE, fp16 accumulator: <=16
                                          terms each <= ~12, exact enough)
        out^T_chunk += v_j^T @ P^T_j     (PE, PSUM accumulate over j)
      PV matmuls lag their S matmul by two tiles (across chunk bounds)
      so exp+mask latency never stalls the PE stream
  - ship out^T [H,T] fp16 (values <~ 10^4, fits) and A [128,T] fp16;
    chunk 3's outputs split into halves on both queues (and the first
    A half as soon as it is final) to shorten the kernel tail
  - host: l = A.sum(axis=0) in fp32, out = (out^T / l).T
"""

import numpy as np

B, T, C, H = 8, 2048, 1024, 128
N_CORES = 8
CHUNK = 512               # q-chunk width (one PSUM bank of fp32)
N_CHUNKS = T // CHUNK     # 4
N_CSUB = C // 128         # 8 contraction subtiles
N_KT = T // 128           # 16 key tiles
KT_PER_CHUNK = CHUNK // 128
SCALE = float(C) ** -0.5  # 1/32, matches reference (embed-size scaling)

W_COLS = N_CSUB * 3 * H          # 3072 fp16 per partition of packed W
X_COLS = N_CSUB * T              # 16384 fp16 per partition of packed x^T
PACK_COLS = W_COLS + X_COLS      # 19456

_CACHE = {}


def _build_bass():
    import concourse.tile as tile
    from concourse import bacc, mybir
    from concourse.masks import make_identity

    fp16 = mybir.dt.float16
    fp32 = mybir.dt.float32
    Exp = mybir.ActivationFunctionType.Exp

    nc = bacc.Bacc("TRN2", target_bir_lowering=False, debug=False,
                   num_devices=N_CORES)

    # host-packed [w | x^T] with per-partition-contiguous, chunk-major
    # layout (see module docstring) so input DMAs use 3-8 KB descriptors
    xTW_d = nc.dram_tensor("xTW", [128, PACK_COLS], fp16,
                           kind="ExternalInput")
    # per-chunk-contiguous output blocks; host reassembles
    outT_d = nc.dram_tensor("outT", [N_CHUNKS, H, CHUNK], fp16,
                            kind="ExternalOutput")
    asum_d = nc.dram_tensor("asum", [N_CHUNKS, 128, CHUNK], fp16,
                            kind="ExternalOutput")

    hwdge = [nc.sync, nc.scalar]  # two HWDGE queues for parallel DMA

    with tile.TileContext(nc) as tc:
        with (
            tc.tile_pool(name="const", bufs=1) as constp,
            tc.tile_pool(name="work", bufs=3) as workp,
            tc.tile_pool(name="pt", bufs=12) as ptp,
        ):
            # warmup operands: a single memset (the very first body op) is
            # the only thing the PE warmup waits for - the warmup matmuls
            # only exist to keep the PE continuously busy from the
            # earliest possible moment
            junk = constp.tile([128, CHUNK], fp16, tag="junk")
            nc.gpsimd.memset(junk[:], 0.0)

            ident = constp.tile([128, 128], fp16, tag="ident")
            make_identity(nc, ident[:])
            # causal-mask bias: negtri[k, q] = -960 where q < k else 0.
            # Accumulated into the diagonal S tiles via one extra 128-row
            # matmul (ident.T @ negtri), it makes exp(S/32 - 30) underflow
            # to exactly 0 in fp16 - removing the separate gpsimd mask
            # from the exp->mask->PV critical chain entirely
            negtri = constp.tile([128, 128], fp16, tag="negtri")
            nc.gpsimd.memset(negtri[:], -960.0)
            # keep -960 where k - q - 1 >= 0 (i.e. q < k), 0 elsewhere
            nc.gpsimd.affine_select(
                out=negtri[:], in_=negtri[:],
                compare_op=mybir.AluOpType.is_ge,
                fill=0.0, base=-1, pattern=[[-1, 128]], channel_multiplier=1)

            # Input DMAs: few large per-partition-contiguous transfers.
            # W is packed per-projection [wv | wq | wk] so the critical
            # set for the first matmuls (wv + x chunk 0) is only 1.13 MB;
            # wq/wk stream in while the v projection runs.  Queue 0
            # (sync) starts ~1.5 us earlier than queue 1 (scalar), so it
            # carries wv and the first x half.
            w_all = constp.tile([128, 3, N_CSUB * H], fp16, tag="w_all")
            # xT free layout is chunk-major: block (ch, c) lives at
            # (ch * N_CSUB + c) * CHUNK
            xT = constp.tile([128, N_CSUB * T], fp16, tag="xT")

            def xslice(ch, c0, c1):
                return slice((ch * N_CSUB + c0) * CHUNK,
                             (ch * N_CSUB + c1) * CHUNK)

            def xdram(ch, c0, c1):
                lo = W_COLS + (ch * N_CSUB + c0) * CHUNK
                return xTW_d.ap()[:, lo:lo + (c1 - c0) * CHUNK]

            WBLK = N_CSUB * H  # 1024 cols per projection's weight block

            def wdram(i):
                return xTW_d.ap()[:, i * WBLK:(i + 1) * WBLK]

            hwdge[0].dma_start(out=w_all[:, 1, :], in_=wdram(1))   # wq
            hwdge[1].dma_start(out=xT[:, xslice(0, 4, 8)],
                               in_=xdram(0, 4, 8))                 # ch0b
            hwdge[0].dma_start(out=xT[:, xslice(0, 0, 4)],
                               in_=xdram(0, 0, 4))                 # ch0a
            hwdge[1].dma_start(out=w_all[:, 0, :], in_=wdram(0))   # wv
            hwdge[0].dma_start(out=w_all[:, 2, :], in_=wdram(2))   # wk
            # later x chunks split across both queues, in need order
            for ch in range(1, N_CHUNKS):
                hwdge[0].dma_start(out=xT[:, xslice(ch, 0, 4)],
                                   in_=xdram(ch, 0, 4))
                hwdge[1].dma_start(out=xT[:, xslice(ch, 4, 8)],
                                   in_=xdram(ch, 4, 8))

            wv = [w_all[:, 0, c * H:(c + 1) * H] for c in range(N_CSUB)]
            wq = [w_all[:, 1, c * H:(c + 1) * H] for c in range(N_CSUB)]
            wk = [w_all[:, 2, c * H:(c + 1) * H] for c in range(N_CSUB)]

            qT = constp.tile([128, T], fp16, tag="qT")
            kT = constp.tile([128, T], fp16, tag="kT")
            vT = constp.tile([128, T], fp16, tag="vT")
            v_nat = constp.tile([128, T], fp16, tag="v_nat")

            # One static PSUM budget for the whole kernel (8 banks exactly)
            # so attention overlaps projections freely.
            with (
                tc.tile_pool(name="pproj", bufs=2, space="PSUM") as psproj,
                tc.tile_pool(name="pvt", bufs=1, space="PSUM") as psvt,
                tc.tile_pool(name="ps_s", bufs=3, space="PSUM") as pss,
                tc.tile_pool(name="ps_o", bufs=2, space="PSUM") as pso,
            ):
                # warm up the PE clock while the input DMAs are in flight;
                # borrow an "o" slot, released long before attention needs
                # it (the PSUM garbage is never read: the first real use
                # of each o bank starts with acc start=True)
                warm_ps = pso.tile([128, CHUNK], fp32, tag="o")
                for _ in range(16):
                    nc.tensor.matmul(warm_ps[:], junk[:, 0:128], junk[:],
                                     start=True, stop=True)

                def tile_geom(ch, j):
                    d = j - ch * KT_PER_CHUNK
                    q0 = ch * CHUNK + (128 * d if d >= 0 else 0)
                    n = (ch + 1) * CHUNK - q0
                    return d, q0, n, q0 - ch * CHUNK

                def attention_s(ch, j):
                    """S matmul + exp + mask + A-accumulate; returns pt."""
                    d, q0, n, lo = tile_geom(ch, j)
                    # for the kernel's LAST two diagonal tiles the
                    # exp->mask->PV chain is tail-exposed: fold the causal
                    # mask into the PSUM group there (one extra 128-row
                    # matmul adding -960 -> exp emits exact fp16 zeros);
                    # everywhere else the gpsimd mask overlaps for free
                    fold = (ch == N_CHUNKS - 1
                            and j >= (ch + 1) * KT_PER_CHUNK - 2)
                    s_ps = pss.tile([128, n], fp32, tag="s")
                    nc.tensor.matmul(s_ps[:], kT[:, j * 128:(j + 1) * 128],
                                     qT[:, q0:(ch + 1) * CHUNK],
                                     start=True, stop=not (d >= 0 and fold))
                    if d >= 0 and fold:
                        nc.tensor.matmul(s_ps[:, 0:128], ident[:],
                                         negtri[:], start=False, stop=True)
                    pt = ptp.tile([128, n], fp16, tag="pt")
                    nc.scalar.activation(pt[:], s_ps[:], Exp, scale=SCALE)
                    if d >= 0 and not fold:
                        # causal triangle on gpsimd: it is otherwise idle,
                        # so the chain never queues behind the DVE's
                        # strict-FIFO A-adds
                        nc.gpsimd.affine_select(
                            out=pt[:, 0:128], in_=pt[:, 0:128],
                            compare_op=mybir.AluOpType.is_ge,
                            fill=0.0, base=0,
                            pattern=[[1, 128]], channel_multiplier=-1)
                    a_sb = a_tiles[ch]
                    if j == 0:
                        nc.vector.tensor_copy(a_sb[:], pt[:])
                    else:
                        nc.vector.tensor_add(a_sb[:, lo:], a_sb[:, lo:],
                                             pt[:])
                    if ch == N_CHUNKS - 1 and d == 1:
                        # a_sb[:, 0:256] is final once the d=1 diagonal
                        # tile's add lands (later tiles start at lo>=256):
                        # ship it now so the tail only carries the rest.
                        # Issued on the sync engine - a DMA issue on the
                        # scalar engine would delay the chain-critical exps
                        nc.sync.dma_start(
                            out=asum_d.ap()[ch][:, 0:CHUNK // 2],
                            in_=a_sb[:, 0:CHUNK // 2])
                    return pt

                def attention_pv(ch, pts, o_ps):
                    n_j = (ch + 1) * KT_PER_CHUNK
                    for j, pt in pts:
                        _, _, _, lo = tile_geom(ch, j)
                        nc.tensor.matmul(o_ps[:, lo:],
                                         v_nat[:, j * 128:(j + 1) * 128],
                                         pt[:],
                                         start=(j == 0), stop=(j == n_j - 1),
                                         skip_group_check=True)

                def attention_out(ch, o_ps):
                    half = CHUNK // 2
                    if ch == N_CHUNKS - 1:
                        # tail path: the first half already shipped when
                        # its PSUM region went final (see emit_pv); copy
                        # the rest via ACT (the DVE still has this chunk's
                        # A-adds queued) and ship on the other queue,
                        # second asum half alongside
                        o_sb = o_sb_hold[ch]
                        nc.scalar.activation(
                            o_sb[:, half:], o_ps[:, half:],
                            mybir.ActivationFunctionType.Copy)
                        nc.scalar.dma_start(out=outT_d.ap()[ch][:, half:],
                                            in_=o_sb[:, half:])
                        nc.sync.dma_start(
                            out=asum_d.ap()[ch][:, half:],
                            in_=a_tiles[ch][:, half:])
                    else:
                        o_sb = workp.tile([128, CHUNK], fp16, tag="osb")
                        nc.vector.tensor_copy(o_sb[:], o_ps[:])
                        hwdge[ch % 2].dma_start(out=outT_d.ap()[ch],
                                                in_=o_sb[:])
                        hwdge[(ch + 1) % 2].dma_start(out=asum_d.ap()[ch],
                                                      in_=a_tiles[ch][:])

                # software-pipelined emission: each PV lags its S by one
                # tile, so the PE stream always has an independent S matmul
                # in front of a PV that might wait on exp; the lag also
                # spans chunk boundaries
                a_tiles = {}
                o_tiles = {}
                o_sb_hold = {}
                pv_count = {}
                pending = []

                def emit_pv(ch, j, pt):
                    n_j = (ch + 1) * KT_PER_CHUNK
                    if pv_count.get(ch, 0) == 0:
                        o_tiles[ch] = pso.tile([128, CHUNK], fp32, tag="o",
                                               name=f"o_ps{ch}")
                    attention_pv(ch, [(j, pt)], o_tiles[ch])
                    pv_count[ch] = pv_count.get(ch, 0) + 1
                    if (ch == N_CHUNKS - 1
                            and pv_count[ch] == n_j - 2):
                        # only the last two diagonal tiles (d=2,3 -> cols
                        # [256:512)) remain, so o_ps[:, 0:256) is final:
                        # copy + ship it now, ~1 us before the last PV
                        half = CHUNK // 2
                        o_sb = workp.tile([128, CHUNK], fp16, tag="osb",
                                          name="osb_last")
                        o_sb_hold[ch] = o_sb
                        nc.scalar.activation(
                            o_sb[:, 0:half], o_tiles[ch][:, 0:half],
                            mybir.ActivationFunctionType.Copy)
                        nc.sync.dma_start(out=outT_d.ap()[ch][:, 0:half],
                                          in_=o_sb[:, 0:half])
                    if pv_count[ch] == n_j:
                        attention_out(ch, o_tiles[ch])

                def attention_seq(ch, js):
                    for j in js:
                        pt = attention_s(ch, j)
                        # lag 2: two independent S matmuls sit between a
                        # PV and the exp+mask chain it depends on
                        if len(pending) >= 2:
                            emit_pv(*pending.pop(0))
                        pending.append((ch, j, pt))

                def proj(ch, w_sb, dstT):
                    cs = slice(ch * CHUNK, (ch + 1) * CHUNK)
                    ps = psproj.tile([128, CHUNK], fp32, tag="proj")
                    for c in range(N_CSUB):
                        nc.tensor.matmul(
                            ps[:], w_sb[c], xT[:, xslice(ch, c, c + 1)],
                            start=(c == 0), stop=(c == N_CSUB - 1))
                    nc.vector.tensor_copy(dstT[:, cs], ps[:])

                # Natural chunk order (chunk 0's attention fills the
                # window while later x chunks stream in).  Within a chunk:
                # q projection first, then the OFF-DIAGONAL attention
                # tiles (they need only qT[ch] and previous chunks'
                # kT/v_nat) so their exps overlap the v/k projections -
                # without this the whole chunk's exp burst lands after the
                # last projection and the ACT engine becomes the
                # bottleneck of the kernel's final stretch.  The diagonal
                # tiles run after k is projected.
                for ch in range(N_CHUNKS):
                    a_tiles[ch] = workp.tile([128, CHUNK], fp16, tag="A",
                                             name=f"a_sb{ch}")
                    proj(ch, wq, qT)
                    n_off = ch * KT_PER_CHUNK
                    # hold back two off-diagonal tiles: interleaved into
                    # the diagonal segment below, their big maskless
                    # matmuls and exps fill the PE while the diagonal
                    # tiles' exp+mask chains resolve (also keeps the vT
                    # copy ahead of the exp-backlogged A-adds in the DVE
                    # FIFO, so the PE transposes aren't delayed)
                    hold = min(2, n_off)
                    # split the off-diagonal run around the v projection
                    # so the vT copy (which gates the PE transposes) sits
                    # mid-queue on the DVE, not behind every A-add
                    n1 = (n_off - hold) // 2
                    attention_seq(ch, range(0, n1))
                    proj(ch, wv, vT)
                    attention_seq(ch, range(n1, n_off - hold))
                    proj(ch, wk, kT)
                    # v natural tiles for this chunk's 4 key tiles
                    for j in range(ch * KT_PER_CHUNK, (ch + 1) * KT_PER_CHUNK):
                        js = slice(j * 128, (j + 1) * 128)
                        psv = psvt.tile([128, 128], fp16, tag="vt",
                                        name=f"psv{j}")
                        nc.tensor.transpose(psv[:], vT[:, js], ident[:])
                        nc.vector.tensor_copy(v_nat[:, js], psv[:])

                    diag = list(range(n_off, n_off + KT_PER_CHUNK))
                    held = list(range(n_off - hold, n_off))
                    attention_seq(ch, [diag[0]] + held + diag[1:])
                while pending:
                    emit_pv(*pending.pop(0))

    nc.compile()
    return nc


def _get_nc():
    if "nc" not in _CACHE:
        _CACHE["nc"] = _build_bass()
    return _CACHE["nc"]


LAST_RESULTS = None


def _pack_inputs(embeddings, Wq, Wk, Wv):
    """Per-core packed [128, PACK_COLS] fp16 arrays (see module docstring)."""
    # per-projection blocks [wv | wq | wk], each [C,H] -> [N_CSUB, 128, H]
    # -> [128, N_CSUB*H] so each block is one per-partition-contiguous DMA
    w_part = np.concatenate(
        [np.asarray(w, dtype=np.float32).astype(np.float16)
         .reshape(N_CSUB, 128, H).transpose(1, 0, 2).reshape(128, N_CSUB * H)
         for w in (Wv, Wq, Wk)], axis=1)          # [128, W_COLS]
    packed = []
    for b in range(B):
        x16 = np.asarray(embeddings[b], dtype=np.float32).astype(np.float16)
        # x^T [C, T] -> [N_CSUB, 128, N_CHUNKS, CHUNK] -> chunk-major
        # [128, N_CHUNKS, N_CSUB, CHUNK] -> [128, X_COLS]
        xp = x16.T.reshape(N_CSUB, 128, N_CHUNKS, CHUNK).transpose(
            1, 2, 0, 3).reshape(128, X_COLS)
        packed.append(np.ascontiguousarray(
            np.concatenate([w_part, xp], axis=1)))
    return packed


def kernel(embeddings: np.ndarray, Wq: np.ndarray, Wk: np.ndarray,
           Wv: np.ndarray) -> np.ndarray:
    from concourse.bass_utils import run_bass_kernel_spmd
    import os

    nc = _get_nc()
    in_maps = [{"xTW": p} for p in _pack_inputs(embeddings, Wq, Wk, Wv)]

    trace = bool(int(os.environ.get("KERNEL_TRACE", "0")))
    res = run_bass_kernel_spmd(nc, in_maps, core_ids=list(range(N_CORES)),
                               trace=trace)
    global LAST_RESULTS
    LAST_RESULTS = res

    out = np.empty((B, T, H), dtype=np.float32)
    for b in range(B):
        # [N_CHUNKS, H, CHUNK] -> [H, T]; denominators from the 128
        # key-partial rows of each chunk's A block (fp16 -> fp32 sum)
        oT = np.concatenate(
            [blk.astype(np.float32) for blk in res.results[b]["outT"]],
            axis=1)
        l = np.concatenate(
            [blk.astype(np.float32).sum(axis=0)
             for blk in res.results[b]["asum"]])
        out[b] = (oT / l[None, :]).T
    return out


# revision 39
# speedup vs baseline: 1.0197x; 1.0197x over previous
"""
Single-head causal attention on 8 Trainium2 NeuronCores.

Problem: embeddings [8, 2048, 1024] fp32, Wq/Wk/Wv [1024, 128] fp32.
    q,k,v = x @ W{q,k,v};  wei = softmax(mask(q k^T * C^-0.5));  out = wei @ v

Sharding: pure data-parallel - one batch element per core, no collectives.

Host-side prep per core (numpy, layout/precision only - all FLOPs stay on
device): cast x and W to fp16 and pack [w | x^T] so that each SBUF
partition's entire input is CONTIGUOUS in DRAM, chunk-major:
    packed[p] = [ wv | wq | wk (1024 each) | x_ch0 | x_ch1 | x_ch2 | x_ch3 ]
This turns every input DMA into 128 descriptors of 2-8 KB (vs 1 KB rows
for a naive x^T load), lifting per-queue DMA throughput from the
~63 GB/s descriptor-rate limit to ~200+ GB/s, so the whole input lands
in ~14 us instead of drip-feeding over 50 us.  The two HWDGE queues are
loaded in consumption order (wq + x chunk 0 first, then wv, wk, and the
remaining x chunks split across both queues).

Per-core device kernel (matmul operands fp16, fp32 PSUM accumulation):
  - 16 PE warmup matmuls on junk SBUF (gated only on one memset) while
    the input DMAs land: the PE pstate ramp + HAM full-clock grant need
    ~4.5 us of CONTINUOUS execution and a gap resets the grant timer,
    so the warmup bridges seamlessly into the first projection
  - per q-chunk ch (natural order 0..3):
      Q^T = Wq^T x^T on PE (N=512 cols, accumulated over C in PSUM),
      then the OFF-DIAGONAL S^T tiles (they need only prior chunks' K/V)
      interleaved around the V^T / K^T projections - this spreads the
      ACT exp load into the projections' shadow; without it the whole
      chunk's exp burst lands after the last projection and ACT becomes
      the bottleneck of the kernel's final stretch
      v natural [T,H] from V^T via 4 PE transposes (128x128 fp16)
      then the 4 diagonal tiles (valid q-range only, N = 512-128*d)
      per 128-key tile j:
        S^T_j = K_j^T.T @ Q^T_chunk      (PE -> PSUM fp32)
        P^T_j = exp(S^T_j / 32)          (ACT, PSUM->SBUF fp16; no
                                          max-sub: |S/32| <~ 2.5, safe)
        causal triangle on the diagonal block (gpsimd affine_select)
        A_chunk += P^T_j                 (DVE, fp16 accumulator: <=16
                                          terms each <= ~12, exact enough)
        out^T_chunk += v_j^T @ P^T_j     (PE, PSUM accumulate over j)
      PV matmuls lag their S matmul by two tiles (across chunk bounds)
      so exp+mask latency never stalls the PE stream
  - ship out^T [H,T] fp16 (values <~ 10^4, fits) and A [128,T] fp16;
    chunk 3's outputs split into halves on both queues (and the first
    A half as soon as it is final) to shorten the kernel tail
  - host: l = A.sum(axis=0) in fp32, out = (out^T / l).T
"""

import numpy as np

B, T, C, H = 8, 2048, 1024, 128
N_CORES = 8
CHUNK = 512               # q-chunk width (one PSUM bank of fp32)
N_CHUNKS = T // CHUNK     # 4
N_CSUB = C // 128         # 8 contraction subtiles
N_KT = T // 128           # 16 key tiles
KT_PER_CHUNK = CHUNK // 128
SCALE = float(C) ** -0.5  # 1/32, matches reference (embed-size scaling)

W_COLS = N_CSUB * 3 * H          # 3072 fp16 per partition of packed W
X_COLS = N_CSUB * T              # 16384 fp16 per partition of packed x^T
PACK_COLS = W_COLS + X_COLS      # 19456

_CACHE = {}


def _build_bass():
    import concourse.tile as tile
    from concourse import bacc, mybir
    from concourse.masks import make_identity

    fp16 = mybir.dt.float16
    fp32 = mybir.dt.float32
    Exp = mybir.ActivationFunctionType.Exp

    nc = bacc.Bacc("TRN2", target_bir_lowering=False, debug=False,
                   num_devices=N_CORES)

    # host-packed [w | x^T] with per-partition-contiguous, chunk-major
    # layout (see module docstring) so input DMAs use 3-8 KB descriptors
    xTW_d = nc.dram_tensor("xTW", [128, PACK_COLS], fp16,
                           kind="ExternalInput")
    # per-chunk-contiguous output blocks; host reassembles
    outT_d = nc.dram_tensor("outT", [N_CHUNKS, H, CHUNK], fp16,
                            kind="ExternalOutput")
    asum_d = nc.dram_tensor("asum", [N_CHUNKS, 128, CHUNK], fp16,
                            kind="ExternalOutput")

    hwdge = [nc.sync, nc.scalar]  # two HWDGE queues for parallel DMA

    with tile.TileContext(nc) as tc:
        with (
            tc.tile_pool(name="const", bufs=1) as constp,
            tc.tile_pool(name="work", bufs=3) as workp,
            tc.tile_pool(name="pt", bufs=12) as ptp,
        ):
            # warmup operands: a single memset (the very first body op) is
            # the only thing the PE warmup waits for - the warmup matmuls
            # only exist to keep the PE continuously busy from the
            # earliest possible moment
            junk = constp.tile([128, CHUNK], fp16, tag="junk")
            nc.gpsimd.memset(junk[:], 0.0)

            ident = constp.tile([128, 128], fp16, tag="ident")
            make_identity(nc, ident[:])


            # Input DMAs: few large per-partition-contiguous transfers.
            # W is packed per-projection [wv | wq | wk] so the critical
            # set for the first matmuls (wv + x chunk 0) is only 1.13 MB;
            # wq/wk stream in while the v projection runs.  Queue 0
            # (sync) starts ~1.5 us earlier than queue 1 (scalar), so it
            # carries wv and the first x half.
            w_all = constp.tile([128, 3, N_CSUB * H], fp16, tag="w_all")
            # xT free layout is chunk-major: block (ch, c) lives at
            # (ch * N_CSUB + c) * CHUNK
            xT = constp.tile([128, N_CSUB * T], fp16, tag="xT")

            def xslice(ch, c0, c1):
                return slice((ch * N_CSUB + c0) * CHUNK,
                             (ch * N_CSUB + c1) * CHUNK)

            def xdram(ch, c0, c1):
                lo = W_COLS + (ch * N_CSUB + c0) * CHUNK
                return xTW_d.ap()[:, lo:lo + (c1 - c0) * CHUNK]

            WBLK = N_CSUB * H  # 1024 cols per projection's weight block

            def wdram(i):
                return xTW_d.ap()[:, i * WBLK:(i + 1) * WBLK]

            hwdge[0].dma_start(out=w_all[:, 1, :], in_=wdram(1))   # wq
            hwdge[1].dma_start(out=xT[:, xslice(0, 4, 8)],
                               in_=xdram(0, 4, 8))                 # ch0b
            hwdge[0].dma_start(out=xT[:, xslice(0, 0, 4)],
                               in_=xdram(0, 0, 4))                 # ch0a
            hwdge[1].dma_start(out=w_all[:, 0, :], in_=wdram(0))   # wv
            hwdge[0].dma_start(out=w_all[:, 2, :], in_=wdram(2))   # wk
            # later x chunks split across both queues, in need order
            for ch in range(1, N_CHUNKS):
                hwdge[0].dma_start(out=xT[:, xslice(ch, 0, 4)],
                                   in_=xdram(ch, 0, 4))
                hwdge[1].dma_start(out=xT[:, xslice(ch, 4, 8)],
                                   in_=xdram(ch, 4, 8))

            wv = [w_all[:, 0, c * H:(c + 1) * H] for c in range(N_CSUB)]
            wq = [w_all[:, 1, c * H:(c + 1) * H] for c in range(N_CSUB)]
            wk = [w_all[:, 2, c * H:(c + 1) * H] for c in range(N_CSUB)]

            qT = constp.tile([128, T], fp16, tag="qT")
            kT = constp.tile([128, T], fp16, tag="kT")
            vT = constp.tile([128, T], fp16, tag="vT")
            v_nat = constp.tile([128, T], fp16, tag="v_nat")

            # One static PSUM budget for the whole kernel (8 banks exactly)
            # so attention overlaps projections freely.
            with (
                tc.tile_pool(name="pproj", bufs=2, space="PSUM") as psproj,
                tc.tile_pool(name="pvt", bufs=1, space="PSUM") as psvt,
                tc.tile_pool(name="ps_s", bufs=3, space="PSUM") as pss,
                tc.tile_pool(name="ps_o", bufs=2, space="PSUM") as pso,
            ):
                # warm up the PE clock while the input DMAs are in flight;
                # borrow an "o" slot, released long before attention needs
                # it (the PSUM garbage is never read: the first real use
                # of each o bank starts with acc start=True)
                warm_ps = pso.tile([128, CHUNK], fp32, tag="o")
                for _ in range(16):
                    nc.tensor.matmul(warm_ps[:], junk[:, 0:128], junk[:],
                                     start=True, stop=True)

                def tile_geom(ch, j):
                    d = j - ch * KT_PER_CHUNK
                    q0 = ch * CHUNK + (128 * d if d >= 0 else 0)
                    n = (ch + 1) * CHUNK - q0
                    return d, q0, n, q0 - ch * CHUNK

                def attention_s(ch, j):
                    """S matmul + exp + mask + A-accumulate; returns pt."""
                    d, q0, n, lo = tile_geom(ch, j)
                    s_ps = pss.tile([128, n], fp32, tag="s")
                    nc.tensor.matmul(s_ps[:], kT[:, j * 128:(j + 1) * 128],
                                     qT[:, q0:(ch + 1) * CHUNK],
                                     start=True, stop=True)
                    pt = ptp.tile([128, n], fp16, tag="pt")
                    nc.scalar.activation(pt[:], s_ps[:], Exp, scale=SCALE)
                    if d >= 0:
                        # causal triangle on gpsimd: it is otherwise idle,
                        # so the exp->mask->PV chain never queues behind
                        # the DVE's strict-FIFO A-adds (folding the mask
                        # into the S PSUM group via a -960 bias matmul was
                        # tried and is NET WORSE: it taxes the PE pipeline)
                        nc.gpsimd.affine_select(
                            out=pt[:, 0:128], in_=pt[:, 0:128],
                            compare_op=mybir.AluOpType.is_ge,
                            fill=0.0, base=0,
                            pattern=[[1, 128]], channel_multiplier=-1)
                    a_sb = a_tiles[ch]
                    if j == 0:
                        nc.vector.tensor_copy(a_sb[:], pt[:])
                    else:
                        nc.vector.tensor_add(a_sb[:, lo:], a_sb[:, lo:],
                                             pt[:])
                    if ch == N_CHUNKS - 1 and d == 1:
                        # a_sb[:, 0:256] is final once the d=1 diagonal
                        # tile's add lands (later tiles start at lo>=256):
                        # ship it now so the tail only carries the rest.
                        # Issued on the sync engine - a DMA issue on the
                        # scalar engine would delay the chain-critical exps
                        nc.sync.dma_start(
                            out=asum_d.ap()[ch][:, 0:CHUNK // 2],
                            in_=a_sb[:, 0:CHUNK // 2])
                    return pt

                def attention_pv(ch, pts, o_ps):
                    n_j = (ch + 1) * KT_PER_CHUNK
                    for j, pt in pts:
                        _, _, _, lo = tile_geom(ch, j)
                        nc.tensor.matmul(o_ps[:, lo:],
                                         v_nat[:, j * 128:(j + 1) * 128],
                                         pt[:],
                                         start=(j == 0), stop=(j == n_j - 1),
                                         skip_group_check=True)

                def attention_out(ch, o_ps):
                    half = CHUNK // 2
                    if ch == N_CHUNKS - 1:
                        # tail path: the first half already shipped when
                        # its PSUM region went final (see emit_pv); copy
                        # the rest via ACT (the DVE still has this chunk's
                        # A-adds queued) and ship on the other queue,
                        # second asum half alongside
                        o_sb = o_sb_hold[ch]
                        nc.scalar.activation(
                            o_sb[:, half:], o_ps[:, half:],
                            mybir.ActivationFunctionType.Copy)
                        nc.scalar.dma_start(out=outT_d.ap()[ch][:, half:],
                                            in_=o_sb[:, half:])
                        nc.sync.dma_start(
                            out=asum_d.ap()[ch][:, half:],
                            in_=a_tiles[ch][:, half:])
                    else:
                        o_sb = workp.tile([128, CHUNK], fp16, tag="osb")
                        nc.vector.tensor_copy(o_sb[:], o_ps[:])
                        hwdge[ch % 2].dma_start(out=outT_d.ap()[ch],
                                                in_=o_sb[:])
                        hwdge[(ch + 1) % 2].dma_start(out=asum_d.ap()[ch],
                                                      in_=a_tiles[ch][:])

                # software-pipelined emission: each PV lags its S by one
                # tile, so the PE stream always has an independent S matmul
                # in front of a PV that might wait on exp; the lag also
                # spans chunk boundaries
                a_tiles = {}
                o_tiles = {}
                o_sb_hold = {}
                pv_count = {}
                pending = []

                def emit_pv(ch, j, pt):
                    n_j = (ch + 1) * KT_PER_CHUNK
                    if pv_count.get(ch, 0) == 0:
                        o_tiles[ch] = pso.tile([128, CHUNK], fp32, tag="o",
                                               name=f"o_ps{ch}")
                    attention_pv(ch, [(j, pt)], o_tiles[ch])
                    pv_count[ch] = pv_count.get(ch, 0) + 1
                    if (ch == N_CHUNKS - 1
                            and pv_count[ch] == n_j - 2):
                        # only the last two diagonal tiles (d=2,3 -> cols
                        # [256:512)) remain, so o_ps[:, 0:256) is final:
                        # copy + ship it now, ~1 us before the last PV
                        half = CHUNK // 2
                        o_sb = workp.tile([128, CHUNK], fp16, tag="osb",
                                          name="osb_last")
                        o_sb_hold[ch] = o_sb
                        nc.scalar.activation(
                            o_sb[:, 0:half], o_tiles[ch][:, 0:half],
                            mybir.ActivationFunctionType.Copy)
                        nc.sync.dma_start(out=outT_d.ap()[ch][:, 0:half],
                                          in_=o_sb[:, 0:half])
                    if pv_count[ch] == n_j:
                        attention_out(ch, o_tiles[ch])

                def attention_seq(ch, js):
                    for j in js:
                        pt = attention_s(ch, j)
                        # lag 2: two independent S matmuls sit between a
                        # PV and the exp+mask chain it depends on
                        if len(pending) >= 2:
                            emit_pv(*pending.pop(0))
                        pending.append((ch, j, pt))

                def proj(ch, w_sb, dstT):
                    cs = slice(ch * CHUNK, (ch + 1) * CHUNK)
                    ps = psproj.tile([128, CHUNK], fp32, tag="proj")
                    for c in range(N_CSUB):
                        nc.tensor.matmul(
                            ps[:], w_sb[c], xT[:, xslice(ch, c, c + 1)],
                            start=(c == 0), stop=(c == N_CSUB - 1))
                    nc.vector.tensor_copy(dstT[:, cs], ps[:])

                # Natural chunk order (chunk 0's attention fills the
                # window while later x chunks stream in).  Within a chunk:
                # q projection first, then the OFF-DIAGONAL attention
                # tiles (they need only qT[ch] and previous chunks'
                # kT/v_nat) so their exps overlap the v/k projections -
                # without this the whole chunk's exp burst lands after the
                # last projection and the ACT engine becomes the
                # bottleneck of the kernel's final stretch.  The diagonal
                # tiles run after k is projected.
                for ch in range(N_CHUNKS):
                    a_tiles[ch] = workp.tile([128, CHUNK], fp16, tag="A",
                                             name=f"a_sb{ch}")
                    proj(ch, wq, qT)
                    n_off = ch * KT_PER_CHUNK
                    # hold back two off-diagonal tiles: interleaved into
                    # the diagonal segment below, their big maskless
                    # matmuls and exps fill the PE while the diagonal
                    # tiles' exp+mask chains resolve (also keeps the vT
                    # copy ahead of the exp-backlogged A-adds in the DVE
                    # FIFO, so the PE transposes aren't delayed)
                    hold = min(2, n_off)
                    # split the off-diagonal run around the v projection
                    # so the vT copy (which gates the PE transposes) sits
                    # mid-queue on the DVE, not behind every A-add
                    n1 = (n_off - hold) // 2
                    attention_seq(ch, range(0, n1))
                    proj(ch, wv, vT)
                    attention_seq(ch, range(n1, n_off - hold))
                    proj(ch, wk, kT)
                    # v natural tiles for this chunk's 4 key tiles
                    for j in range(ch * KT_PER_CHUNK, (ch + 1) * KT_PER_CHUNK):
                        js = slice(j * 128, (j + 1) * 128)
                        psv = psvt.tile([128, 128], fp16, tag="vt",
                                        name=f"psv{j}")
                        nc.tensor.transpose(psv[:], vT[:, js], ident[:])
                        nc.vector.tensor_copy(v_nat[:, js], psv[:])

                    diag = list(range(n_off, n_off + KT_PER_CHUNK))
                    held = list(range(n_off - hold, n_off))
                    attention_seq(ch, [diag[0]] + held + diag[1:])
                while pending:
                    emit_pv(*pending.pop(0))

    nc.compile()
    return nc


def _get_nc():
    if "nc" not in _CACHE:
        _CACHE["nc"] = _build_bass()
    return _CACHE["nc"]


LAST_RESULTS = None


def _pack_inputs(embeddings, Wq, Wk, Wv):
    """Per-core packed [128, PACK_COLS] fp16 arrays (see module docstring)."""
    # per-projection blocks [wv | wq | wk], each [C,H] -> [N_CSUB, 128, H]
    # -> [128, N_CSUB*H] so each block is one per-partition-contiguous DMA
    w_part = np.concatenate(
        [np.asarray(w, dtype=np.float32).astype(np.float16)
         .reshape(N_CSUB, 128, H).transpose(1, 0, 2).reshape(128, N_CSUB * H)
         for w in (Wv, Wq, Wk)], axis=1)          # [128, W_COLS]
    packed = []
    for b in range(B):
        x16 = np.asarray(embeddings[b], dtype=np.float32).astype(np.float16)
        # x^T [C, T] -> [N_CSUB, 128, N_CHUNKS, CHUNK] -> chunk-major
        # [128, N_CHUNKS, N_CSUB, CHUNK] -> [128, X_COLS]
        xp = x16.T.reshape(N_CSUB, 128, N_CHUNKS, CHUNK).transpose(
            1, 2, 0, 3).reshape(128, X_COLS)
        packed.append(np.ascontiguousarray(
            np.concatenate([w_part, xp], axis=1)))
    return packed


def kernel(embeddings: np.ndarray, Wq: np.ndarray, Wk: np.ndarray,
           Wv: np.ndarray) -> np.ndarray:
    from concourse.bass_utils import run_bass_kernel_spmd
    import os

    nc = _get_nc()
    in_maps = [{"xTW": p} for p in _pack_inputs(embeddings, Wq, Wk, Wv)]

    trace = bool(int(os.environ.get("KERNEL_TRACE", "0")))
    res = run_bass_kernel_spmd(nc, in_maps, core_ids=list(range(N_CORES)),
                               trace=trace)
    global LAST_RESULTS
    LAST_RESULTS = res

    out = np.empty((B, T, H), dtype=np.float32)
    for b in range(B):
        # [N_CHUNKS, H, CHUNK] -> [H, T]; denominators from the 128
        # key-partial rows of each chunk's A block (fp16 -> fp32 sum)
        oT = np.concatenate(
            [blk.astype(np.float32) for blk in res.results[b]["outT"]],
            axis=1)
        l = np.concatenate(
            [blk.astype(np.float32).sum(axis=0)
             for blk in res.results[b]["asum"]])
        out[b] = (oT / l[None, :]).T
    return out
